# revision 82
# baseline (speedup 1.0000x reference)
"""Trainium2 Bass kernel for nn_CrossAttention_72275709657317.

Reference computation (B=4, S=2048, E=1024, D=64):
    Q = x @ Wq.T + bq                      [B,S,D]
    K = y @ Wk.T + bk                      [B,S,D]
    scores = Q @ K.T / sqrt(D)             [B,Sq,Sk]
    attn = softmax(scores, axis=1)         (softmax over the QUERY axis)
    V = (y @ WvR.T + bvR) @ WvL.T + bvL    [B,S,E]
    out = attn @ V                         [B,S,E]

Algebraic restructuring:
  * V is rank-64 (+bias): attn @ V = (attn @ [VR | 1]) @ [[WvL.T],[bvL]].
  * softmax over q: attn[q,k] = exp(s[q,k])/den[k], den[k] = sum_q exp.
    den is folded into the VR' rows; attnT stays unnormalized.

Implementation notes (v10, ~65-69us HW vs the 110us fp32r v1):
  * bf16 end-to-end (fro rel err ~2.9e-3 vs the 2e-2 gate); PSUM
    accumulation stays fp32.
  * ALL loads are plain DMACopies on the single SP HWDGE queue: x/y and
    the weight packs are pre-transposed on the host into the device
    layout (partition e%128, chunk e//128), block-contiguous per 512-col
    half so each DMA is 128 runs of 8KB.  One instruction kind
    everywhere: the tile framework serializes DMAs across the two HWDGE
    queues AND across kinds (DmaTranspose vs DMACopy) at ~2.5us per
    transition, while same-kind same-queue DMAs pipeline freely.
  * Weights/biases are packed on the host into two bf16 arrays (fused
    [Wk|WvR] chunks + duplicated Wq chunks + WvLT with bvL row + f32
    biases bitcast into bf16 pairs) -> 2 XBAR loads, no setup compute.
  * Inputs stream in arrival-interleaved order (y0, x0, y1, x1) and the
    projection blocks are emitted interleaved with the scores tiles they
    unlock, so the exp stream starts ~4us before the last input lands.
    The exp stream runs all 16 local-k tiles first so the kv exchange
    latency fully hides.
  * VR^T -> VRu runs on the PE (identity transpose), NOT the XBAR: the
    list scheduler hoists data-ready DmaTransposes into the kv/den
    DMACopy chains, paying uncontrollable kind-transition stalls.
  * exp runs on the Act engine (the only engine with activations,
    ~612ns/512-tile); den partials ride the DVE (tensor_reduce), the
    pair K/VR subtraction rides the GpSimd engine so neither blocks the
    other's queue.
  * den pair-exchange is split into two half-chains with an asymmetric
    reconstruction: denf[remote] needs only the EARLY chain (hidden
    mid-stream), so O1T starts its remote-kc matmuls while the late
    chain is still in flight (kc_order remote-first).

Sharding: 8 cores -> (batch b = c//2, query-half h = c%2).  Pairwise
AllReduce exchanges K^T/VR^T and den with "partner = pair_sum - mine".
"""
import numpy as np
import ml_dtypes

import concourse.bass as bass
import concourse.tile as tile
from concourse import bacc, mybir
from concourse.bass_utils import run_bass_kernel_spmd
from concourse.masks import make_identity

N_CORES = 8
B, S, E, D = 4, 2048, 1024, 64
H = S // 2            # per-core q rows / local k rows
P = 128
EB = E // P           # 8 e-chunks
KC = S // P           # 16 k-chunks
KCL = H // P          # 8 local k-chunks
NQ = H // 512         # 2 q-chunks of 512
DV = D + 1            # VR plus folded-ones column
F32 = mybir.dt.float32
BF16 = mybir.dt.bfloat16
EXP = mybir.ActivationFunctionType.Exp
ADD = mybir.AluOpType.add
GROUPS = [[0, 1], [2, 3], [4, 5], [6, 7]]

# wpackA rows: 1024 = WkvT (vstack(Wk, WvR) per 128-col e-chunk), 2 = bias_kv
# (f32 viewed as bf16 pairs), pad -> 1040 (multiple of 16).
# wpackB rows: 1024 = WqqT (Wq duplicated per e-chunk), 2 = bias_q,
# 1024 = WvLT' row j = [WvL[j,:] | bvL[j] | pad], pad -> 2064.
WA = 1040
WB = 2064
IN_SPECS = [
    ("xT", [NQ, P, EB, 512], BF16), ("yT", [NQ, P, EB, 512], BF16),
    ("wpackA", [P, WA], BF16), ("wpackB", [P, WB], BF16),
]


def _emit(tc, aps, out_ap, no_cc=False, stop_stage=99):
    nc = tc.nc
    from contextlib import ExitStack
    with ExitStack() as ctx:
        const = ctx.enter_context(tc.tile_pool(name="const", bufs=1))
        work = ctx.enter_context(tc.tile_pool(name="work", bufs=4))
        big = ctx.enter_context(tc.tile_pool(name="big", bufs=1))
        mm_ps = ctx.enter_context(tc.tile_pool(name="mm_ps", bufs=4, space="PSUM"))
        tp_ps = ctx.enter_context(tc.tile_pool(name="tp_ps", bufs=2, space="PSUM"))
        o1_ps = ctx.enter_context(tc.tile_pool(name="o1_ps", bufs=2, space="PSUM"))
        dram = ctx.enter_context(tc.tile_pool(name="dram", bufs=1, space="DRAM"))

        # ------- one XBAR stream: packed weights, then inputs -------------
        # All DMAs are XBAR transposes on the SP queue: same-kind HWDGE DMAs
        # pipeline freely, while any DmaTranspose<->DMACopy transition gets a
        # serializing sem from the tile framework (~2.5us each).  The weight
        # packs are host-pre-transposed so they can ride the same XBAR path.
        # y loads first: KTVR_l gates the exp stream (every scores tile needs
        # a KT chunk as lhsT) and also feeds the 3-hop kv exchange; x only
        # supplies the rhs (QT), whose second half isn't needed until tile 9.
        wA = const.tile([P, WA], BF16, name="wA")
        wB = const.tile([P, WB], BF16, name="wB")
        yT = big.tile([P, EB, H], BF16, name="yT")
        xT = big.tile([P, EB, H], BF16, name="xT")
        nc.sync.dma_start(wA[:], aps["wpackA"])
        nc.sync.dma_start(yT[:, :, 0:512], aps["yT"][0])
        nc.sync.dma_start(xT[:, :, 0:512], aps["xT"][0])
        nc.sync.dma_start(wB[:], aps["wpackB"])
        nc.sync.dma_start(yT[:, :, 512:H], aps["yT"][1])
        nc.sync.dma_start(xT[:, :, 512:H], aps["xT"][1])

        WkvT = wA[:, 0:E].rearrange("p (a b) -> p a b", a=EB)
        bias_kv = wA[:, E:E + 2].bitcast(F32)
        WqqT = wB[:, 0:E].rearrange("p (a b) -> p a b", a=EB)
        bias_q = wB[0:D, E:E + 2].bitcast(F32)
        WvLT = wB[0:DV, E + 2:2 * E + 2]

        # ---------------- persistent tiles ----------------
        KTVR_l = big.tile([P, H], BF16, name="KTVR_l")  # rows 0:64 K^T, 64:128 VR^T
        KTVR_r = big.tile([P, H], BF16, name="KTVR_r")
        QT = big.tile([D, H], BF16, name="QT")
        attnT = big.tile([P, KC, H], BF16, name="attnT")
        VRu = big.tile([P, KC, D], BF16, name="VRu")
        VRp = big.tile([P, KC, P], BF16, name="VRp")
        nc.gpsimd.memset(VRp[:, :, DV:P], 0.0)
        identb = const.tile([P, P], BF16, name="identb")
        make_identity(nc, identb[:])

        kv_loc = [dram.tile([P, 512], BF16, name=f"kv_loc{kb}") for kb in range(NQ)]
        kv_sum = [dram.tile([P, 512], BF16, name=f"kv_sum{kb}") for kb in range(NQ)]
        den_dram = [dram.tile([P, KCL], F32, name=f"den_dram{i}") for i in range(2)]
        den_sum_dram = [dram.tile([P, KCL], F32, name=f"den_sum{i}") for i in range(2)]

        if stop_stage <= 0:
            nc.sync.dma_start(out_ap[0:P, 0:P], WkvT[:, 0, :])
            return

        # ------- projections + scores, interleaved by DMA arrival -------
        # PE/Act run their queues in issue order, so emission follows the
        # input arrival order (y0, x0, y1, x1): each projection block is
        # followed by the scores tiles it unlocks, letting the exp stream
        # start ~5us before the last input lands.
        den_loc = big.tile([P, KC], F32, name="den_loc")
        den2 = big.tile([P, KC, NQ], F32, name="den2")
        dsum = big.tile([P, KC], F32, name="dsum")      # [sumL | sumR]
        kvs = big.tile([P, H], BF16, name="kvs")

        def proj_y(kb):
            ssl = slice(kb * 512, (kb + 1) * 512)
            ps = mm_ps.tile([P, 512], F32, name="mmps")
            for ei in range(EB):
                nc.tensor.matmul(ps[:], WkvT[:, ei, :], yT[:, ei, ssl],
                                 start=(ei == 0), stop=(ei == EB - 1))
            nc.scalar.add(KTVR_l[:, ssl], ps[:], bias_kv)

        def proj_x(qb):
            ssl = slice(qb * 512, (qb + 1) * 512)
            ps = mm_ps.tile([P, 512], F32, name="mmps")
            for ei in range(EB):
                nc.tensor.matmul(ps[:], WqqT[:, ei, :], xT[:, ei, ssl],
                                 start=(ei == 0), stop=(ei == EB - 1))
            nc.scalar.add(QT[:, ssl], ps[0:D, :], bias_q)

        def scores(qc, kcs):
            qsl = slice(qc * 512, (qc + 1) * 512)
            for kc in kcs:
                ktv = KTVR_l if kc < KCL else KTVR_r
                col = (kc if kc < KCL else kc - KCL) * P
                sps = mm_ps.tile([P, 512], F32, name="mmps")
                nc.tensor.matmul(sps[:], ktv[0:D, col:col + P], QT[:, qsl],
                                 start=True, stop=True)
                nc.scalar.activation(attnT[:, kc, qsl], sps[:], EXP,
                                     scale=0.125)
                nc.vector.tensor_reduce(den2[:, kc, qc:qc + 1],
                                        attnT[:, kc, qsl],
                                        axis=mybir.AxisListType.X, op=ADD)

        def vru_half(half):
            # VR^T rows -> VRu [k%128, k//128, d] on the PE (transpose via
            # identity).  NOT an XBAR DMA: any DmaTranspose between the kv /
            # den DMACopy chains gets a serializing kind-transition sem from
            # the scheduler, which we can't control by emission order.
            ktv = KTVR_l if half == 0 else KTVR_r
            for j in range(KCL):
                kc = half * KCL + j
                tp = tp_ps.tile([P, D], BF16, name="tpps")
                nc.tensor.transpose(tp[:], ktv[D:P, j * P:(j + 1) * P],
                                    identb[D:P, D:P])
                nc.vector.tensor_copy(VRu[:, kc, :], tp[:])

        def den_chain(i):
            hsl = slice(i * KCL, (i + 1) * KCL)
            nc.vector.tensor_reduce(den_loc[:, hsl], den2[:, hsl, :],
                                    axis=mybir.AxisListType.X, op=ADD)
            nc.sync.dma_start(den_dram[i][:], den_loc[:, hsl])
            if no_cc:
                nc.sync.dma_start(den_sum_dram[i][:], den_dram[i][:])
            else:
                nc.gpsimd.collective_compute(
                    "AllReduce", ADD, replica_groups=GROUPS,
                    ins=[den_dram[i].opt()], outs=[den_sum_dram[i].opt()])
            nc.sync.dma_start(dsum[:, hsl], den_sum_dram[i][:])

        proj_y(0)
        proj_x(0)
        scores(0, range(0, 4))
        proj_y(1)
        scores(0, range(4, KCL))
        proj_x(1)

        # kv exchange chains, split per kb half.  The exp stream still has
        # 16+ local tiles queued when KTVR_r lands.
        for kb in range(NQ):
            ssl = slice(kb * 512, (kb + 1) * 512)
            loc, summ = kv_loc[kb], kv_sum[kb]
            nc.sync.dma_start(loc[:], KTVR_l[:, ssl])
            if no_cc:
                nc.sync.dma_start(summ[:], loc[:])
            else:
                nc.gpsimd.collective_compute(
                    "AllReduce", ADD, replica_groups=GROUPS,
                    ins=[loc.opt()], outs=[summ.opt()])
            nc.sync.dma_start(kvs[:, ssl], summ[:])
            nc.gpsimd.tensor_sub(KTVR_r[:, ssl], kvs[:, ssl], KTVR_l[:, ssl])

        if stop_stage <= 1:
            nc.sync.dma_start(out_ap[0:P, 0:H], KTVR_l[:])
            return
        if stop_stage <= 2:
            nc.sync.dma_start(out_ap[0:D, 0:H], QT[:])
            return

        scores(1, range(0, KCL))
        den_chain(0)                      # local half: hides under remote exp
        vru_half(0)
        scores(0, range(KCL, KC))
        vru_half(1)
        scores(1, range(KCL, KC))
        den_chain(1)                      # late half: O1T overlaps its flight

        if stop_stage <= 3:
            nc.sync.dma_start(out_ap[0:P, 0:H], attnT[:, 0, :])
            return

        denf = big.tile([P, KC], F32, name="denf")
        r_sb = big.tile([P, KC], F32, name="r_sb")
        partner = big.tile([P, KC], F32, name="partner")
        for mine, other in ((slice(KCL, KC), slice(0, KCL)),
                            (slice(0, KCL), slice(KCL, KC))):
            nc.vector.tensor_sub(partner[:, other], dsum[:, other],
                                 den_loc[:, other])
            nc.vector.tensor_add(denf[:, mine], den_loc[:, mine],
                                 partner[:, other])
            nc.vector.reciprocal(r_sb[:, mine], denf[:, mine])
            # VR' = [VR * r | r | 0-pad] for this half
            for kc in range(mine.start, mine.stop):
                nc.vector.tensor_scalar_mul(VRp[:, kc, 0:D], VRu[:, kc, :],
                                            r_sb[:, kc:kc + 1])
                nc.vector.tensor_copy(VRp[:, kc, D:DV], r_sb[:, kc:kc + 1])

        if stop_stage <= 4:
            nc.sync.dma_start(out_ap[0:P, 0:P], VRp[:, 0, :])
            return

        # ------- O1T = VR'^T @ attnT, then out = O1T^T @ WvLT, per qc -----
        # kc order remote-first: remote VRp is ready ~7us before local (its
        # den half only needs the early collective), so the chain starts
        # while the late den chain is still in flight.
        O1T = big.tile([DV, H], BF16, name="O1T")
        kc_order = list(range(KCL, KC)) + list(range(0, KCL))
        for qc in range(NQ):
            qsl = slice(qc * 512, (qc + 1) * 512)
            ops_ = o1_ps.tile([P, 512], F32, name="o1ps")
            for j, kc in enumerate(kc_order):
                nc.tensor.matmul(ops_[:], VRp[:, kc, :], attnT[:, kc, qsl],
                                 start=(j == 0), stop=(j == KC - 1))
            nc.scalar.copy(O1T[:, qsl], ops_[0:DV, :])
            for qp in range(qc * 2, (qc + 1) * 2):
                ot = work.tile([P, 2, E], BF16, name="outt")
                for j in range(2):
                    qo = qp * 2 + j
                    for vc in range(2):
                        fps = mm_ps.tile([P, 512], F32, name="mmps")
                        nc.tensor.matmul(fps[:],
                                         O1T[:, qo * P:(qo + 1) * P],
                                         WvLT[:, vc * 512:(vc + 1) * 512],
                                         start=True, stop=True)
                        if vc == 0:
                            nc.scalar.copy(ot[:, j, 0:512], fps[:])
                        else:
                            nc.vector.tensor_copy(ot[:, j, 512:E], fps[:])
                nc.sync.dma_start(
                    out_ap[qp * 2 * P:(qp + 1) * 2 * P, :]
                    .rearrange("(c p) e -> p c e", p=P), ot[:])


def build_nc(reps: int = 1, no_cc=False, stop_stage=99):
    nc = bacc.Bacc("TRN2", target_bir_lowering=False, debug=False,
                   num_devices=N_CORES)
    aps = {name: nc.dram_tensor(name, shape, dt, kind="ExternalInput").ap()
           for name, shape, dt in IN_SPECS}
    out_ap = nc.dram_tensor("out", [H, E], BF16, kind="ExternalOutput").ap()
    with tile.TileContext(nc) as tc:
        if reps == 1:
            _emit(tc, aps, out_ap, no_cc=no_cc, stop_stage=stop_stage)
        else:
            with tc.For_i(0, reps, 1):
                _emit(tc, aps, out_ap, no_cc=no_cc, stop_stage=stop_stage)
    nc.compile()
    return nc


def prep_weights(inputs):
    """Host-side packed weight layouts, shared by all cores.

    The packs are stored pre-transposed ([rows, 128]) so the device loads
    them through the same XBAR DMA-transpose path as x/y; on chip they land
    as [128, rows].  Row r of wpackA/B supplies free-offset r:
      wpackA: r in [0,1024): WkvT  -> fused [Wk | WvR] e-chunk transposed;
              r in [1024,1026): bias_kv (f32 [128] viewed as bf16 pairs);
              r in [1026,1028): bias_q;  rest zero pad.
      wpackB: r in [0,1024): WqqT (Wq duplicated);
              r in [1024,2048): WvLT' row j = [WvL[j,:] | bvL[j] | pad].
    """
    bf = ml_dtypes.bfloat16
    f = {k: np.asarray(v, dtype=np.float32) for k, v in inputs.items()}
    A = np.zeros((WA, P), bf)
    Bp = np.zeros((WB, P), bf)
    # (built row-major as before, transposed to the direct [128, rows]
    # device layout on return)
    for ei in range(EB):
        esl = slice(ei * P, (ei + 1) * P)
        A[ei * P:(ei + 1) * P, :] = np.concatenate(
            [f["Wk"][:, esl], f["WvR"][:, esl]], axis=0).astype(bf)
        Bp[ei * P:(ei + 1) * P, :] = np.concatenate(
            [f["Wq"][:, esl], f["Wq"][:, esl]], axis=0).astype(bf)
    bias_kv = np.concatenate([f["bk"], f["bvR"]]).astype(np.float32)  # [128]
    A[E:E + 2, :] = bias_kv.view(bf).reshape(P, 2).T
    bias_q = f["bq"].astype(np.float32)                               # [64]
    Bp[E:E + 2, 0:D] = bias_q.view(bf).reshape(D, 2).T
    Bp[E + 2:2 * E + 2, 0:DV] = np.concatenate(
        [f["WvL"], f["bvL"][:, None]], axis=1).astype(bf)
    return {"wpackA": np.ascontiguousarray(A.T),
            "wpackB": np.ascontiguousarray(Bp.T)}


def prep_input(shard_bf16):
    # [H, E] -> [sb, p, ei, 512]: element (s, e) lands at partition e%128,
    # chunk e//128, col s (the XBAR transpose mapping, done on the host).
    # Block-contiguous per 512-col half so each device DMA reads one 8KB
    # run per partition instead of 1024 strided 1KB descriptors.
    t = shard_bf16.T.reshape(EB, P, NQ, 512)    # [ei, p, sb, s']
    return np.ascontiguousarray(t.transpose(2, 1, 0, 3))


def make_in_maps(inputs):
    bf = ml_dtypes.bfloat16
    w = prep_weights(inputs)
    xb = np.asarray(inputs["x"], dtype=np.float32).astype(bf)
    yb = np.asarray(inputs["y"], dtype=np.float32).astype(bf)
    in_maps = []
    for c in range(N_CORES):
        b, h = divmod(c, 2)
        m = {"xT": prep_input(xb[b, h * H:(h + 1) * H, :]),
             "yT": prep_input(yb[b, h * H:(h + 1) * H, :])}
        m.update(w)
        in_maps.append(m)
    return in_maps


def assemble_out(results):
    out = np.empty((B, S, E), dtype=np.float32)
    for c in range(N_CORES):
        b, h = divmod(c, 2)
        out[b, h * H:(h + 1) * H, :] = results[c]["out"].astype(np.float32)
    return out


_NC = None


def kernel(**inputs) -> np.ndarray:
    global _NC
    if _NC is None:
        _NC = build_nc()
    in_maps = make_in_maps(inputs)
    res = run_bass_kernel_spmd(_NC, in_maps, list(range(N_CORES)))
    return assemble_out(res.results)


# revision 83
# speedup vs baseline: 1.0833x; 1.0833x over previous
"""Trainium2 Bass kernel for nn_CrossAttention_72275709657317.

Reference computation (B=4, S=2048, E=1024, D=64):
    Q = x @ Wq.T + bq                      [B,S,D]
    K = y @ Wk.T + bk                      [B,S,D]
    scores = Q @ K.T / sqrt(D)             [B,Sq,Sk]
    attn = softmax(scores, axis=1)         (softmax over the QUERY axis)
    V = (y @ WvR.T + bvR) @ WvL.T + bvL    [B,S,E]
    out = attn @ V                         [B,S,E]

Algebraic restructuring:
  * V is rank-64 (+bias): attn @ V = (attn @ [VR | 1]) @ [[WvL.T],[bvL]].
  * softmax over q: attn[q,k] = exp(s[q,k])/den[k], den[k] = sum_q exp.
    den is folded into the VR' rows; attnT stays unnormalized.

Implementation notes (v10, ~65-69us HW vs the 110us fp32r v1):
  * bf16 end-to-end (fro rel err ~2.9e-3 vs the 2e-2 gate); PSUM
    accumulation stays fp32.
  * ALL loads are plain DMACopies on the single SP HWDGE queue: x/y and
    the weight packs are pre-transposed on the host into the device
    layout (partition e%128, chunk e//128), block-contiguous per 512-col
    half so each DMA is 128 runs of 8KB.  One instruction kind
    everywhere: the tile framework serializes DMAs across the two HWDGE
    queues AND across kinds (DmaTranspose vs DMACopy) at ~2.5us per
    transition, while same-kind same-queue DMAs pipeline freely.
  * Weights/biases are packed on the host into two bf16 arrays (fused
    [Wk|WvR] chunks + duplicated Wq chunks + WvLT with bvL row + f32
    biases bitcast into bf16 pairs) -> 2 XBAR loads, no setup compute.
  * Inputs stream in arrival-interleaved order (y0, x0, y1, x1) and the
    projection blocks are emitted interleaved with the scores tiles they
    unlock, so the exp stream starts ~4us before the last input lands.
    The exp stream runs all 16 local-k tiles first so the kv exchange
    latency fully hides.
  * VR^T -> VRu runs on the PE (identity transpose), NOT the XBAR: the
    list scheduler hoists data-ready DmaTransposes into the kv/den
    DMACopy chains, paying uncontrollable kind-transition stalls.
  * exp runs on the Act engine (the only engine with activations,
    ~612ns/512-tile); den partials ride the DVE (tensor_reduce), the
    pair K/VR subtraction rides the GpSimd engine so neither blocks the
    other's queue.
  * den pair-exchange is split into two half-chains with an asymmetric
    reconstruction: denf[remote] needs only the EARLY chain (hidden
    mid-stream), so O1T starts its remote-kc matmuls while the late
    chain is still in flight (kc_order remote-first).

Sharding: 8 cores -> (batch b = c//2, query-half h = c%2).  Pairwise
AllReduce exchanges K^T/VR^T and den with "partner = pair_sum - mine".
"""
import numpy as np
import ml_dtypes

import concourse.bass as bass
import concourse.tile as tile
from concourse import bacc, mybir
from concourse.bass_utils import run_bass_kernel_spmd
from concourse.masks import make_identity

N_CORES = 8
B, S, E, D = 4, 2048, 1024, 64
H = S // 2            # per-core q rows / local k rows
P = 128
EB = E // P           # 8 e-chunks
KC = S // P           # 16 k-chunks
KCL = H // P          # 8 local k-chunks
NQ = H // 512         # 2 q-chunks of 512
DV = D + 1            # VR plus folded-ones column
F32 = mybir.dt.float32
BF16 = mybir.dt.bfloat16
EXP = mybir.ActivationFunctionType.Exp
ADD = mybir.AluOpType.add
GROUPS = [[0, 1], [2, 3], [4, 5], [6, 7]]

# wpackA rows: 1024 = WkvT (vstack(Wk, WvR) per 128-col e-chunk), 2 = bias_kv
# (f32 viewed as bf16 pairs), pad -> 1040 (multiple of 16).
# wpackB rows: 1024 = WqqT (Wq duplicated per e-chunk), 2 = bias_q,
# 1024 = WvLT' row j = [WvL[j,:] | bvL[j] | pad], pad -> 2064.
WA = 1040
WB = 2064
IN_SPECS = [
    ("xT", [NQ, P, EB, 512], BF16), ("yT", [NQ, P, EB, 512], BF16),
    ("wpackA", [P, WA], BF16), ("wpackB", [P, WB], BF16),
]


def _emit(tc, aps, out_ap, no_cc=False, stop_stage=99):
    nc = tc.nc
    from contextlib import ExitStack
    with ExitStack() as ctx:
        const = ctx.enter_context(tc.tile_pool(name="const", bufs=1))
        work = ctx.enter_context(tc.tile_pool(name="work", bufs=4))
        big = ctx.enter_context(tc.tile_pool(name="big", bufs=1))
        mm_ps = ctx.enter_context(tc.tile_pool(name="mm_ps", bufs=4, space="PSUM"))
        tp_ps = ctx.enter_context(tc.tile_pool(name="tp_ps", bufs=2, space="PSUM"))
        o1_ps = ctx.enter_context(tc.tile_pool(name="o1_ps", bufs=2, space="PSUM"))
        dram = ctx.enter_context(tc.tile_pool(name="dram", bufs=1, space="DRAM"))

        # ------- one XBAR stream: packed weights, then inputs -------------
        # All DMAs are XBAR transposes on the SP queue: same-kind HWDGE DMAs
        # pipeline freely, while any DmaTranspose<->DMACopy transition gets a
        # serializing sem from the tile framework (~2.5us each).  The weight
        # packs are host-pre-transposed so they can ride the same XBAR path.
        # y loads first: KTVR_l gates the exp stream (every scores tile needs
        # a KT chunk as lhsT) and also feeds the 3-hop kv exchange; x only
        # supplies the rhs (QT), whose second half isn't needed until tile 9.
        wA = const.tile([P, WA], BF16, name="wA")
        wB = const.tile([P, WB], BF16, name="wB")
        yT = big.tile([P, EB, H], BF16, name="yT")
        xT = big.tile([P, EB, H], BF16, name="xT")
        nc.sync.dma_start(wA[:], aps["wpackA"])
        nc.sync.dma_start(yT[:, :, 0:512], aps["yT"][0])
        nc.sync.dma_start(xT[:, :, 0:512], aps["xT"][0])
        nc.sync.dma_start(wB[:], aps["wpackB"])
        nc.sync.dma_start(yT[:, :, 512:H], aps["yT"][1])
        nc.sync.dma_start(xT[:, :, 512:H], aps["xT"][1])

        WkvT = wA[:, 0:E].rearrange("p (a b) -> p a b", a=EB)
        bias_kv = wA[:, E:E + 2].bitcast(F32)
        WqqT = wB[:, 0:E].rearrange("p (a b) -> p a b", a=EB)
        bias_q = wB[0:D, E:E + 2].bitcast(F32)
        WvLT = wB[0:DV, E + 2:2 * E + 2]

        # ---------------- persistent tiles ----------------
        KTVR_l = big.tile([P, H], BF16, name="KTVR_l")  # rows 0:64 K^T, 64:128 VR^T
        KTVR_r = big.tile([P, H], BF16, name="KTVR_r")
        QT = big.tile([D, H], BF16, name="QT")
        attnT = big.tile([P, KC, H], BF16, name="attnT")
        VRu = big.tile([P, KC, D], BF16, name="VRu")
        VRp = big.tile([P, KC, P], BF16, name="VRp")
        nc.gpsimd.memset(VRp[:, :, DV:P], 0.0)
        identb = const.tile([P, P], BF16, name="identb")
        make_identity(nc, identb[:])

        kv_loc = [dram.tile([P, 512], BF16, name=f"kv_loc{kb}") for kb in range(NQ)]
        kv_sum = [dram.tile([P, 512], BF16, name=f"kv_sum{kb}") for kb in range(NQ)]
        den_dram = [dram.tile([P, KCL], F32, name=f"den_dram{i}") for i in range(2)]
        den_sum_dram = [dram.tile([P, KCL], F32, name=f"den_sum{i}") for i in range(2)]

        if stop_stage <= 0:
            nc.sync.dma_start(out_ap[0:P, 0:P], WkvT[:, 0, :])
            return

        # ------- projections + scores, interleaved by DMA arrival -------
        # PE/Act run their queues in issue order, so emission follows the
        # input arrival order (y0, x0, y1, x1): each projection block is
        # followed by the scores tiles it unlocks, letting the exp stream
        # start ~5us before the last input lands.
        den_loc = big.tile([P, KC], F32, name="den_loc")
        den2 = big.tile([P, KC, NQ], F32, name="den2")
        dsum = big.tile([P, KC], F32, name="dsum")      # [sumL | sumR]
        kvs = big.tile([P, H], BF16, name="kvs")

        def proj_y(kb):
            ssl = slice(kb * 512, (kb + 1) * 512)
            ps = mm_ps.tile([P, 512], F32, name="mmps")
            for ei in range(EB):
                nc.tensor.matmul(ps[:], WkvT[:, ei, :], yT[:, ei, ssl],
                                 start=(ei == 0), stop=(ei == EB - 1))
            nc.scalar.add(KTVR_l[:, ssl], ps[:], bias_kv)

        def proj_x(qb):
            ssl = slice(qb * 512, (qb + 1) * 512)
            ps = mm_ps.tile([P, 512], F32, name="mmps")
            for ei in range(EB):
                nc.tensor.matmul(ps[:], WqqT[:, ei, :], xT[:, ei, ssl],
                                 start=(ei == 0), stop=(ei == EB - 1))
            nc.scalar.add(QT[:, ssl], ps[0:D, :], bias_q)

        def scores(qc, kcs):
            qsl = slice(qc * 512, (qc + 1) * 512)
            for kc in kcs:
                ktv = KTVR_l if kc < KCL else KTVR_r
                col = (kc if kc < KCL else kc - KCL) * P
                sps = mm_ps.tile([P, 512], F32, name="mmps")
                nc.tensor.matmul(sps[:], ktv[0:D, col:col + P], QT[:, qsl],
                                 start=True, stop=True)
                nc.scalar.activation(attnT[:, kc, qsl], sps[:], EXP,
                                     scale=0.125)
                nc.vector.tensor_reduce(den2[:, kc, qc:qc + 1],
                                        attnT[:, kc, qsl],
                                        axis=mybir.AxisListType.X, op=ADD)

        def vru_half(half):
            # VR^T rows -> VRu [k%128, k//128, d] on the PE (transpose via
            # identity).  NOT an XBAR DMA: any DmaTranspose between the kv /
            # den DMACopy chains gets a serializing kind-transition sem from
            # the scheduler, which we can't control by emission order.
            ktv = KTVR_l if half == 0 else KTVR_r
            for j in range(KCL):
                kc = half * KCL + j
                tp = tp_ps.tile([P, D], BF16, name="tpps")
                nc.tensor.transpose(tp[:], ktv[D:P, j * P:(j + 1) * P],
                                    identb[D:P, D:P])
                nc.vector.tensor_copy(VRu[:, kc, :], tp[:])

        def den_chain(i):
            hsl = slice(i * KCL, (i + 1) * KCL)
            nc.vector.tensor_reduce(den_loc[:, hsl], den2[:, hsl, :],
                                    axis=mybir.AxisListType.X, op=ADD)
            nc.sync.dma_start(den_dram[i][:], den_loc[:, hsl])
            if no_cc:
                nc.sync.dma_start(den_sum_dram[i][:], den_dram[i][:])
            else:
                nc.gpsimd.collective_compute(
                    "AllReduce", ADD, replica_groups=GROUPS,
                    ins=[den_dram[i].opt()], outs=[den_sum_dram[i].opt()])
            nc.sync.dma_start(dsum[:, hsl], den_sum_dram[i][:])

        proj_y(0)
        proj_x(0)
        scores(0, range(0, 4))
        proj_y(1)
        scores(0, range(4, KCL))
        proj_x(1)

        # kv exchange chains, split per kb half.  The exp stream still has
        # 16+ local tiles queued when KTVR_r lands.
        for kb in range(NQ):
            ssl = slice(kb * 512, (kb + 1) * 512)
            loc, summ = kv_loc[kb], kv_sum[kb]
            nc.sync.dma_start(loc[:], KTVR_l[:, ssl])
            if no_cc:
                nc.sync.dma_start(summ[:], loc[:])
            else:
                nc.gpsimd.collective_compute(
                    "AllReduce", ADD, replica_groups=GROUPS,
                    ins=[loc.opt()], outs=[summ.opt()])
            nc.sync.dma_start(kvs[:, ssl], summ[:])
            nc.gpsimd.tensor_sub(KTVR_r[:, ssl], kvs[:, ssl], KTVR_l[:, ssl])

        if stop_stage <= 1:
            nc.sync.dma_start(out_ap[0:P, 0:H], KTVR_l[:])
            return
        if stop_stage <= 2:
            nc.sync.dma_start(out_ap[0:D, 0:H], QT[:])
            return

        scores(1, range(0, KCL))
        den_chain(0)                      # local half: hides under remote exp
        vru_half(0)
        scores(0, range(KCL, KC))
        vru_half(1)
        scores(1, range(KCL, KC))
        den_chain(1)                      # late half: O1T overlaps its flight

        if stop_stage <= 3:
            nc.sync.dma_start(out_ap[0:P, 0:H], attnT[:, 0, :])
            return

        denf = big.tile([P, KC], F32, name="denf")
        r_sb = big.tile([P, KC], F32, name="r_sb")
        partner = big.tile([P, KC], F32, name="partner")
        for mine, other in ((slice(KCL, KC), slice(0, KCL)),
                            (slice(0, KCL), slice(KCL, KC))):
            nc.vector.tensor_sub(partner[:, other], dsum[:, other],
                                 den_loc[:, other])
            nc.vector.tensor_add(denf[:, mine], den_loc[:, mine],
                                 partner[:, other])
            nc.vector.reciprocal(r_sb[:, mine], denf[:, mine])
            # VR' = [VR * r | r | 0-pad] for this half
            for kc in range(mine.start, mine.stop):
                nc.vector.tensor_scalar_mul(VRp[:, kc, 0:D], VRu[:, kc, :],
                                            r_sb[:, kc:kc + 1])
                nc.vector.tensor_copy(VRp[:, kc, D:DV], r_sb[:, kc:kc + 1])

        if stop_stage <= 4:
            nc.sync.dma_start(out_ap[0:P, 0:P], VRp[:, 0, :])
            return

        # ------- O1T = VR'^T @ attnT, then out = O1T^T @ WvLT, per qc -----
        # kc order remote-first: remote VRp is ready ~7us before local (its
        # den half only needs the early collective), so the chain starts
        # while the late den chain is still in flight.
        O1T = big.tile([DV, H], BF16, name="O1T")
        kc_order = list(range(KCL, KC)) + list(range(0, KCL))
        for qc in range(NQ):
            qsl = slice(qc * 512, (qc + 1) * 512)
            ops_ = o1_ps.tile([P, 512], F32, name="o1ps")
            for j, kc in enumerate(kc_order):
                nc.tensor.matmul(ops_[:], VRp[:, kc, :], attnT[:, kc, qsl],
                                 start=(j == 0), stop=(j == KC - 1))
            nc.scalar.copy(O1T[:, qsl], ops_[0:DV, :])
            for qo in range(qc * 4, (qc + 1) * 4):
                ot = work.tile([P, E], BF16, name="outt")
                for vc in range(2):
                    fps = mm_ps.tile([P, 512], F32, name="mmps")
                    nc.tensor.matmul(fps[:],
                                     O1T[:, qo * P:(qo + 1) * P],
                                     WvLT[:, vc * 512:(vc + 1) * 512],
                                     start=True, stop=True)
                    if vc == 0:
                        nc.scalar.copy(ot[:, 0:512], fps[:])
                    else:
                        nc.vector.tensor_copy(ot[:, 512:E], fps[:])
                nc.sync.dma_start(out_ap[qo * P:(qo + 1) * P, :], ot[:])


def build_nc(reps: int = 1, no_cc=False, stop_stage=99):
    nc = bacc.Bacc("TRN2", target_bir_lowering=False, debug=False,
                   num_devices=N_CORES)
    aps = {name: nc.dram_tensor(name, shape, dt, kind="ExternalInput").ap()
           for name, shape, dt in IN_SPECS}
    out_ap = nc.dram_tensor("out", [H, E], BF16, kind="ExternalOutput").ap()
    with tile.TileContext(nc) as tc:
        if reps == 1:
            _emit(tc, aps, out_ap, no_cc=no_cc, stop_stage=stop_stage)
        else:
            with tc.For_i(0, reps, 1):
                _emit(tc, aps, out_ap, no_cc=no_cc, stop_stage=stop_stage)
    nc.compile()
    return nc


def prep_weights(inputs):
    """Host-side packed weight layouts, shared by all cores.

    The packs are stored pre-transposed ([rows, 128]) so the device loads
    them through the same XBAR DMA-transpose path as x/y; on chip they land
    as [128, rows].  Row r of wpackA/B supplies free-offset r:
      wpackA: r in [0,1024): WkvT  -> fused [Wk | WvR] e-chunk transposed;
              r in [1024,1026): bias_kv (f32 [128] viewed as bf16 pairs);
              r in [1026,1028): bias_q;  rest zero pad.
      wpackB: r in [0,1024): WqqT (Wq duplicated);
              r in [1024,2048): WvLT' row j = [WvL[j,:] | bvL[j] | pad].
    """
    bf = ml_dtypes.bfloat16
    f = {k: np.asarray(v, dtype=np.float32) for k, v in inputs.items()}
    A = np.zeros((WA, P), bf)
    Bp = np.zeros((WB, P), bf)
    # (built row-major as before, transposed to the direct [128, rows]
    # device layout on return)
    for ei in range(EB):
        esl = slice(ei * P, (ei + 1) * P)
        A[ei * P:(ei + 1) * P, :] = np.concatenate(
            [f["Wk"][:, esl], f["WvR"][:, esl]], axis=0).astype(bf)
        Bp[ei * P:(ei + 1) * P, :] = np.concatenate(
            [f["Wq"][:, esl], f["Wq"][:, esl]], axis=0).astype(bf)
    bias_kv = np.concatenate([f["bk"], f["bvR"]]).astype(np.float32)  # [128]
    A[E:E + 2, :] = bias_kv.view(bf).reshape(P, 2).T
    bias_q = f["bq"].astype(np.float32)                               # [64]
    Bp[E:E + 2, 0:D] = bias_q.view(bf).reshape(D, 2).T
    Bp[E + 2:2 * E + 2, 0:DV] = np.concatenate(
        [f["WvL"], f["bvL"][:, None]], axis=1).astype(bf)
    return {"wpackA": np.ascontiguousarray(A.T),
            "wpackB": np.ascontiguousarray(Bp.T)}


def prep_input(shard_bf16):
    # [H, E] -> [sb, p, ei, 512]: element (s, e) lands at partition e%128,
    # chunk e//128, col s (the XBAR transpose mapping, done on the host).
    # Block-contiguous per 512-col half so each device DMA reads one 8KB
    # run per partition instead of 1024 strided 1KB descriptors.
    t = shard_bf16.T.reshape(EB, P, NQ, 512)    # [ei, p, sb, s']
    return np.ascontiguousarray(t.transpose(2, 1, 0, 3))


def make_in_maps(inputs):
    bf = ml_dtypes.bfloat16
    w = prep_weights(inputs)
    xb = np.asarray(inputs["x"], dtype=np.float32).astype(bf)
    yb = np.asarray(inputs["y"], dtype=np.float32).astype(bf)
    in_maps = []
    for c in range(N_CORES):
        b, h = divmod(c, 2)
        m = {"xT": prep_input(xb[b, h * H:(h + 1) * H, :]),
             "yT": prep_input(yb[b, h * H:(h + 1) * H, :])}
        m.update(w)
        in_maps.append(m)
    return in_maps


def assemble_out(results):
    out = np.empty((B, S, E), dtype=np.float32)
    for c in range(N_CORES):
        b, h = divmod(c, 2)
        out[b, h * H:(h + 1) * H, :] = results[c]["out"].astype(np.float32)
    return out


_NC = None


def kernel(**inputs) -> np.ndarray:
    global _NC
    if _NC is None:
        _NC = build_nc()
    in_maps = make_in_maps(inputs)
    res = run_bass_kernel_spmd(_NC, in_maps, list(range(N_CORES)))
    return assemble_out(res.results)
